# revision 30
# baseline (speedup 1.0000x reference)
"""Trainium2 Bass kernel for the 3-block invertible coupling flow (RealNVP-style).

Computation (per sample row of x = [u1(256) | u2(256) | t(1)]):
    for j in 3 blocks:
        v1 = u1 * exp(mlp_s2(u2)) + mlp_t2(u2)
        v2 = u2 * exp(mlp_s1(v1)) + mlp_t1(v1)
        u1, u2 = v1, v2
    out = [u1 | u2 | t]
Each mlp is 256 -> 32 (tanh) -> 256.

v4 strategy (pure data parallel over batch, 131072 -> 8 cores x 16384):
  * Host-side batch transpose: each core gets x_t [512, bc] feature-major and
    returns out [512, bc]; the t column never touches the device.
  * All matmuls float32r (1 cycle/row when the PE is warm).  Hidden units are
    duplicated [s,s,t,t] in the first layer so each chain's K=32 second-layer
    matmuls come from its own 32-row groups: with chains A/B assigned groups
    (0,64) and (32,96), all four run concurrently via tile_position.
  * TWO batch chains form a "pair" sharing one instruction per elementwise
    stage (instruction overhead, not streaming, dominates ACT/DVE):
      - L1 writes ph_pair [128, 2ch, 512] (chain = bank); ONE tanh instr;
      - L2 s/t outputs land in [128, 2ch, 512] psum tiles per fc;
      - exp fc0 (bias=b2s) and fc1 (bias-free; exp(b2s) folded into the fc1
        multiply) are ONE ACT instr each;
      - fc0 multiply: GPSIMD tensor_mul [128,2,512] (SBUF only);
      - fc1 multiply: DVE scalar_tensor_tensor (u*eb2s)*ee;
      - readout per fc: ONE DVE scalar_tensor_tensor (tmp+b2t)+t_psum over
        [128, 2ch, 512] (b2t is per-feature, so identical across chains).
  * PSUM: tag "h" [128,2,512] bufs=1 (2 banks) + tag "st" bufs=3 (6) = 8.
"""

from contextlib import ExitStack

import numpy as np

import concourse.bass as bass
import concourse.tile as tile
from concourse import bacc, mybir
from concourse.bass_utils import run_bass_kernel_spmd

F32 = mybir.dt.float32
F32R = mybir.dt.float32r

USE_F32R = True

B_TOTAL = 131072
D = 512
S = 256
H = 32
L = 3
NCORES = 8
BT = 512  # batch columns per chain-tile (= one PSUM bank of fp32)

MMDT = F32R if USE_F32R else F32


def _f32(ap):
    """View a float32r AP as plain float32 for non-matmul consumers."""
    return ap.bitcast(F32) if USE_F32R else ap


def _pack_weights(W1, b1, W2, b2):
    """Host-side repack of the MLP weights (hidden duplicated [s,s,t,t]).

    q=0 updates u1 from u2 (s-idx 1, t-idx 3); q=1 updates u2 from v1
    (s-idx 0, t-idx 2).
    """
    W1 = np.asarray(W1, np.float32)
    b1 = np.asarray(b1, np.float32)
    W2 = np.asarray(W2, np.float32)
    b2 = np.asarray(b2, np.float32)
    w1p = np.empty((L, 2, 2, 128, 128), np.float32)
    b1p = np.empty((L, 2, 128), np.float32)
    w2p = np.empty((L, 2, 128, 256), np.float32)
    b2sp = np.empty((L, 2, 128), np.float32)    # exp bias for fc0
    eb2sp = np.empty((L, 2, 128), np.float32)   # exp(b2s) scale for fc1
    b2tp = np.empty((L, 2, 128, 2), np.float32)
    for j in range(L):
        for q in range(2):
            s_idx, t_idx = (1, 3) if q == 0 else (0, 2)
            for c in range(2):
                blk = slice(c * 128, (c + 1) * 128)
                w1p[j, q, c, :, 0:32] = W1[j, s_idx, blk, :]
                w1p[j, q, c, :, 32:64] = W1[j, s_idx, blk, :]
                w1p[j, q, c, :, 64:96] = W1[j, t_idx, blk, :]
                w1p[j, q, c, :, 96:128] = W1[j, t_idx, blk, :]
            b1p[j, q, 0:32] = b1[j, s_idx]
            b1p[j, q, 32:64] = b1[j, s_idx]
            b1p[j, q, 64:96] = b1[j, t_idx]
            b1p[j, q, 96:128] = b1[j, t_idx]
            w2p[j, q, 0:32, :] = W2[j, s_idx]
            w2p[j, q, 32:64, :] = W2[j, s_idx]
            w2p[j, q, 64:96, :] = W2[j, t_idx]
            w2p[j, q, 96:128, :] = W2[j, t_idx]
            b2sp[j, q] = b2[j, s_idx, 0:128]
            eb2sp[j, q] = np.exp(b2[j, s_idx, 128:256])
            b2tp[j, q, :, 0] = b2[j, t_idx, 0:128]
            b2tp[j, q, :, 1] = b2[j, t_idx, 128:256]
    return dict(w1p=w1p, b1p=b1p, w2p=w2p, b2sp=b2sp, eb2sp=eb2sp, b2tp=b2tp)


def build_nc(bc):
    """Per-core Bass program; x_t [512, bc] feature-major in, out [512, bc]."""
    assert bc % (2 * BT) == 0
    npair = bc // (2 * BT)
    nc = bacc.Bacc(None, target_bir_lowering=False)
    x_d = nc.declare_dram_parameter("x_t", [D, bc], MMDT, isOutput=False)
    w1_d = nc.declare_dram_parameter("w1p", [L, 2, 2, 128, 128], MMDT, isOutput=False)
    b1_d = nc.declare_dram_parameter("b1p", [L, 2, 128], F32, isOutput=False)
    w2_d = nc.declare_dram_parameter("w2p", [L, 2, 128, 256], MMDT, isOutput=False)
    b2s_d = nc.declare_dram_parameter("b2sp", [L, 2, 128], F32, isOutput=False)
    eb2s_d = nc.declare_dram_parameter("eb2sp", [L, 2, 128], F32, isOutput=False)
    b2t_d = nc.declare_dram_parameter("b2tp", [L, 2, 128, 2], F32, isOutput=False)
    out_d = nc.declare_dram_parameter("out", [D, bc], MMDT, isOutput=True)

    TANH = mybir.ActivationFunctionType.Tanh
    EXP = mybir.ActivationFunctionType.Exp
    ADD = mybir.AluOpType.add
    MULT = mybir.AluOpType.mult

    with tile.TileContext(nc) as tc, ExitStack() as ctx:
        singles = ctx.enter_context(tc.tile_pool(name="singles", bufs=1))
        p_state = ctx.enter_context(tc.tile_pool(name="state", bufs=4))
        p_th = ctx.enter_context(tc.tile_pool(name="th", bufs=3))
        p_e = ctx.enter_context(tc.tile_pool(name="e", bufs=3))
        p_tmp = ctx.enter_context(tc.tile_pool(name="tmp", bufs=3))
        ps_h = ctx.enter_context(
            tc.tile_pool(name="ps_h", bufs=2, space=bass.MemorySpace.PSUM)
        )
        ps_st = ctx.enter_context(
            tc.tile_pool(name="ps_st", bufs=2, space=bass.MemorySpace.PSUM)
        )

        # --- weights (persist in SBUF) -----------------------------------
        w1s = singles.tile([128, L, 2, 2, 128], MMDT)
        nc.gpsimd.dma_start(
            out=w1s[:], in_=w1_d[:].rearrange("j q c p m -> p j q c m")
        )
        b1s = singles.tile([128, L, 2], F32)
        nc.gpsimd.dma_start(out=b1s[:], in_=b1_d[:].rearrange("j q p -> p j q"))
        w2s = singles.tile([128, L, 2, 256], MMDT)
        nc.gpsimd.dma_start(
            out=w2s[:], in_=w2_d[:].rearrange("j q p m -> p j q m")
        )
        b2ss = singles.tile([128, L, 2], F32)
        nc.gpsimd.dma_start(out=b2ss[:], in_=b2s_d[:].rearrange("j q p -> p j q"))
        eb2ss = singles.tile([128, L, 2], F32)
        nc.gpsimd.dma_start(out=eb2ss[:], in_=eb2s_d[:].rearrange("j q p -> p j q"))
        b2ts = singles.tile([128, L, 2, 2], F32)
        nc.gpsimd.dma_start(out=b2ts[:], in_=b2t_d[:].rearrange("j q p c -> p j q c"))

        assert npair % 2 == 0
        for sup in range(npair // 2):
            prs = (2 * sup, 2 * sup + 1)
            # paired state tiles: [128, chain(2), fc(2), BT]; two pairs (four
            # chains) are interleaved per half-step so every engine queue
            # always holds the other pair's independent, ready instructions
            us = {}
            for pr in prs:
                b0 = pr * 2 * BT
                u = []
                for h in range(2):
                    ut = p_state.tile(
                        [128, 2, 2, BT], MMDT, tag=f"st{h}{pr % 2}", name=f"ut{h}{pr}"
                    )
                    for ch in range(2):
                        bch = b0 + ch * BT
                        nc.sync.dma_start(
                            out=ut[:, ch, :, :],
                            in_=x_d[h * S : (h + 1) * S, bch : bch + BT].rearrange(
                                "(c p) b -> p c b", p=128
                            ),
                        )
                    u.append(ut)
                us[pr] = u

            for j in range(L):
                for q in range(2):
                  for pr in prs:
                    u = us[pr]
                    hin = u[1 - q]
                    tgt = u[q]
                    # L1 per chain (M=128, duplicated hidden), shared psum tile
                    ph = ps_h.tile([128, 2, BT], F32, tag="h")
                    for ch in range(2):
                        for c in range(2):
                            nc.tensor.matmul(
                                ph[:, ch, :],
                                w1s[:, j, q, c, :],
                                hin[:, ch, c, :],
                                start=(c == 0),
                                stop=(c == 1),
                            )
                    th = p_th.tile([128, 2, BT], MMDT, tag="th")
                    nc.scalar.activation(
                        th[:], ph[:], TANH, bias=b1s[:, j, q : q + 1]
                    )
                    # L2: chains use disjoint row groups via the duplication:
                    # A reads its rows (0,64), B its rows (32,96)
                    pss = {}
                    pst = {}
                    for fc in range(2):
                        pss[fc] = ps_st.tile(
                            [128, 2, BT], F32, tag="st", name=f"pss{fc}"
                        )
                        for ch in range(2):
                            r = 32 * ch
                            nc.tensor.matmul(
                                pss[fc][:, ch, :],
                                w2s[r : r + 32, j, q, fc * 128 : (fc + 1) * 128],
                                th[r : r + 32, ch, :],
                                tile_position=(r, 0),
                            )
                    ee = p_e.tile([128, 2, 2, BT], F32, tag="e")
                    nc.scalar.activation(
                        ee[:, :, 0, :], pss[0][:], EXP, bias=b2ss[:, j, q : q + 1]
                    )
                    nc.scalar.activation(ee[:, :, 1, :], pss[1][:], EXP)
                    # fc0 multiply on GPSIMD (plain; b2s went through exp
                    # bias), split per chain so the first readout can start
                    # after ~half the GPSIMD work
                    tmp0 = p_tmp.tile([128, 2, BT], F32, tag="tmp0")
                    for ch in range(2):
                        nc.gpsimd.tensor_mul(
                            out=tmp0[:, ch, :],
                            in0=_f32(tgt[:, ch, 0, :]),
                            in1=ee[:, ch, 0, :],
                        )
                    # fc1 multiply on DVE with exp(b2s) folded in
                    tmp1 = p_tmp.tile([128, 2, BT], MMDT, tag="tmp1")
                    nc.vector.scalar_tensor_tensor(
                        out=tmp1[:],
                        in0=_f32(tgt[:, :, 1, :]),
                        scalar=eb2ss[:, j, q : q + 1],
                        in1=ee[:, :, 1, :],
                        op0=MULT,
                        op1=MULT,
                    )
                    tmp = {0: tmp0, 1: tmp1}
                    v = p_state.tile(
                        [128, 2, 2, BT], MMDT, tag=f"st{q}{pr % 2}", name=f"v{pr}"
                    )
                    # t matmuls AFTER the multiplies: the t psum banks are
                    # then held only across the readout, not the whole
                    # exp/multiply chain, so independent pairs can overlap
                    for fc in range(2):
                        pst[fc] = ps_st.tile(
                            [128, 2, BT], F32, tag="st", name=f"pst{fc}"
                        )
                        for ch in range(2):
                            r = 64 + 32 * ch
                            nc.tensor.matmul(
                                pst[fc][:, ch, :],
                                w2s[r : r + 32, j, q, fc * 128 : (fc + 1) * 128],
                                th[r : r + 32, ch, :],
                                tile_position=(r, 0),
                            )
                        if fc == 0:
                            # per-chain readouts chase the split GPSIMD muls
                            for ch in range(2):
                                nc.vector.scalar_tensor_tensor(
                                    out=v[:, ch, 0, :],
                                    in0=_f32(tmp0[:, ch, :]),
                                    scalar=b2ts[:, j, q, 0:1],
                                    in1=pst[0][:, ch, :],
                                    op0=ADD,
                                    op1=ADD,
                                )
                        else:
                            nc.vector.scalar_tensor_tensor(
                                out=v[:, :, 1, :],
                                in0=_f32(tmp1[:]),
                                scalar=b2ts[:, j, q, 1:2],
                                in1=pst[1][:],
                                op0=ADD,
                                op1=ADD,
                            )
                    u[q] = v

            for pr in prs:
                b0 = pr * 2 * BT
                for h in range(2):
                    for ch in range(2):
                        bch = b0 + ch * BT
                        nc.sync.dma_start(
                            out=out_d[h * S : (h + 1) * S, bch : bch + BT].rearrange(
                                "(c p) b -> p c b", p=128
                            ),
                            in_=us[pr][h][:, ch, :, :],
                        )
    nc.compile()
    return nc


_NC_CACHE = {}
TRACE = False
LAST_EXEC_NS = None
LAST_RES = None


def _get_nc(bc):
    if bc not in _NC_CACHE:
        _NC_CACHE[bc] = build_nc(bc)
    return _NC_CACHE[bc]


def kernel(x, W1, b1, W2, b2):
    global LAST_EXEC_NS
    x = np.asarray(x, np.float32)
    b = x.shape[0]
    assert b % NCORES == 0
    bc = b // NCORES
    packed = _pack_weights(W1, b1, W2, b2)
    nc = _get_nc(bc)
    in_maps = [
        {
            "x_t": np.ascontiguousarray(x[i * bc : (i + 1) * bc, :D].T),
            **packed,
        }
        for i in range(NCORES)
    ]
    res = run_bass_kernel_spmd(nc, in_maps, list(range(NCORES)), trace=TRACE)
    if getattr(res, "exec_time_ns", None):
        LAST_EXEC_NS = res.exec_time_ns
    if TRACE:
        globals()["LAST_RES"] = res
    out = np.empty((b, D + 1), np.float32)
    for i in range(NCORES):
        out[i * bc : (i + 1) * bc, :D] = res.results[i]["out"].T
    out[:, D] = x[:, D]
    return out
